# revision 8
# baseline (speedup 1.0000x reference)
"""Trainium2 Bass kernel for nn_ActionSmoothingLoss.

Math (per row y of previous_actions, x = segmented log_softmax(current_action)):
    e = exp(y)                       (no max-subtraction: |y| <= ~5.5, safe in f32)
    Z_j = sum_{i in seg j} e_i
    S_j = sum_{i in seg j} e_i * (y_i - x_i)
    loss = (1/W) * sum_rows sum_j inv_n_j * [ S_j / Z_j - log Z_j ]

Sharding: data-parallel over W across 8 cores; x replicated; partial sums
(per-partition accumulators) gathered and combined on host.

Device pipeline per tile [128 partitions, r=64 rows x 68]:
    ScalarE: e = exp(y); per-segment Ln(Z) with accum into accB.
    DVE:     d = y - xbb (subtract)
             cums = MUL_CUMSUM(e, d)   <- custom DVE op: prefix-sum of the
                    product e*d in one pass (fuses multiply + S-reduction)
             smp  = cums sampled at the 6 segment-end columns of each row
             S    = adjacent-difference of smp  (exact per-(row,seg) sums:
                    the cumsum is continuous across rows, so diffs of
                    consecutive segment-end samples telescope correctly)
             Z    = 6 per-segment tensor_reduces over e
             rz   = reciprocal_approx_fast(Z)
             stt  = (S * inv_n) * rz with accum into accA (4 groups)
Final combine of accA/accB on host in float64.
"""

import sys

sys.path.insert(0, "/opt/trn_rl_repo")

import numpy as np

NVEC = (3, 3, 4, 25, 25, 8)
OFFS = (0, 3, 6, 10, 35, 60)
ENDS = (2, 5, 9, 34, 59, 67)  # inclusive end column of each segment
A = 68
P = 128
N_CORES = 8
W_FULL = 524288
W_CORE = W_FULL // N_CORES  # 65536
R = 64                      # rows per partition per tile
F = R * A                   # 4352 free elems per tile
T = W_CORE // (P * R)       # 8 tiles per core

_PROGRAM_CACHE = {}
_MUL_CUMSUM = None


def _register_mul_cumsum():
    """Register the MUL_CUMSUM_ANT custom DVE op (out = cumsum(in0*in1) along
    the free dim, fp32 state). Uses the documented extension point
    (dve_ops.OPS); the uop table ships inside the NEFF so no firmware change
    is involved. Idempotent."""
    global _MUL_CUMSUM
    if _MUL_CUMSUM is not None:
        return _MUL_CUMSUM
    import concourse.dve_ops as dve_ops_mod
    from concourse.dve_spec import Spec, Src0, Src1, AluOp, scan, lower
    from concourse.dve_uop import DveOpSpec

    NAME = "MUL_CUMSUM_ANT"
    for op in dve_ops_mod.OPS:
        if op.name == NAME:
            _MUL_CUMSUM = op
            return op

    def _ref(in0, in1, s0, s1, imm2):
        p = in0.shape[0]
        prod = (np.asarray(in0, np.float32).reshape(p, -1)
                * np.asarray(in1, np.float32).reshape(p, -1)).astype(np.float32)
        return np.cumsum(prod, axis=-1, dtype=np.float32)

    spec = Spec(body=scan(AluOp.ADD, Src0 * Src1), reference=_ref)
    row = dve_ops_mod._CUSTOM_DVE_ROW_BASE + len(dve_ops_mod.OPS)
    assert row < 0x20
    shas = {}
    for ver in ("v3",):
        s = DveOpSpec(name=NAME, opcode=row, uops=lower(spec, ver=ver), rd1_en=True)
        shas[ver] = s.sha(ver)
    op = dve_ops_mod.DveOp(NAME, spec, subdim=False, uops_sha=shas)
    dve_ops_mod.OPS.append(op)
    dve_ops_mod._SUB_OPCODE_FOR_NAME[NAME] = row
    dve_ops_mod.CUSTOM_DVE_SPECS[NAME] = spec
    _MUL_CUMSUM = op
    return op


def build_program(w_core=W_CORE, r=R):
    import concourse.bass as bass
    import concourse.bacc as bacc
    import concourse.mybir as mybir
    from concourse import tile

    mul_cumsum = _register_mul_cumsum()

    f32 = mybir.dt.float32
    Ft = r * A
    S6 = 6 * r
    Tt = w_core // (P * r)
    assert Tt * P * r == w_core

    Exp = mybir.ActivationFunctionType.Exp
    Ln = mybir.ActivationFunctionType.Ln
    sub_op = mybir.AluOpType.subtract
    mult_op = mybir.AluOpType.mult
    add_op = mybir.AluOpType.add
    AX = mybir.AxisListType.X

    nc = bacc.Bacc(None, target_bir_lowering=False)
    pa = nc.dram_tensor("pa", [w_core, A], f32, kind="ExternalInput")
    # xb carries x broadcast (cols 0..67) plus the 6 inv_n values (68..73).
    xb = nc.dram_tensor("xb", [P, A + 6], f32, kind="ExternalInput")
    acc_a = nc.dram_tensor("acc_a", [P, Tt], f32, kind="ExternalOutput")
    acc_b = nc.dram_tensor("acc_b", [P, Tt * 6], f32, kind="ExternalOutput")

    pav = pa.rearrange("(t p r) a -> t p (r a)", t=Tt, p=P, r=r)

    with tile.TileContext(nc) as tc:
        with tc.tile_pool(name="io", bufs=3) as io, \
             tc.tile_pool(name="wk", bufs=2) as wk, \
             tc.tile_pool(name="sm", bufs=2) as sm, \
             tc.tile_pool(name="ps", bufs=1) as ps:
            xbt = ps.tile([P, A + 6], f32)
            nc.sync.dma_start(xbt[:], xb[:])
            accA = ps.tile([P, Tt], f32)
            accB = ps.tile([P, Tt * 6], f32)
            # x broadcast to [P, r*A] once (sub's in1 per half).
            xbb = ps.tile([P, Ft], f32)
            nc.vector.tensor_copy(
                xbb[:].rearrange("p (r a) -> p r a", r=r),
                xbt[:, :A].unsqueeze(1).broadcast_to((P, r, A)))
            # inv_n broadcast to the [P, r, 6] j-innermost layout once.
            invbb = ps.tile([P, 6 * r], f32)
            nc.vector.tensor_copy(
                invbb[:].rearrange("p (r s) -> p r s", s=6),
                xbt[:, A:A + 6].unsqueeze(1).broadcast_to((P, r, 6)))
            H = Ft // 2
            for t in range(Tt):
                y = io.tile([P, Ft], f32, tag="y")
                nc.sync.dma_start(y[:, :H], pav[t][:, :H])
                nc.sync.dma_start(y[:, H:], pav[t][:, H:])
                e = wk.tile([P, Ft], f32, tag="e")
                d = wk.tile([P, Ft], f32, tag="d")
                cums = wk.tile([P, Ft], f32, tag="cums")
                for h in (slice(0, H), slice(H, Ft)):
                    nc.scalar.activation(e[:, h], y[:, h], Exp)
                    nc.vector.tensor_tensor(d[:, h], y[:, h], xbb[:, h], op=sub_op)
                e3 = e[:].rearrange("p (r a) -> p r a", r=r)
                d3 = d[:].rearrange("p (r a) -> p r a", r=r)
                # cums = running sum of e*d over the flat [r*A] stream.
                nc.vector._custom_dve(
                    mul_cumsum, out=cums[:], in0=e3, in1=d3)
                cums3 = cums[:].rearrange("p (r a) -> p r a", r=r)
                # Sample the cumsum at each segment-end column; j-innermost
                # layout so one adjacent-difference yields every segment sum.
                # End cols {2,5}, {9,34,59}, {67} have affine strides, so three
                # strided ScalarE copies cover all six (ScalarE has slack).
                smp = sm.tile([P, S6], f32, tag="smp")
                smp3 = smp[:].rearrange("p (r s) -> p r s", s=6)
                nc.scalar.copy(smp3[:, :, 0:2], cums3[:, :, 2:6:3])
                nc.scalar.copy(smp3[:, :, 2:5], cums3[:, :, 9:60:25])
                nc.scalar.copy(smp3[:, :, 5:6], cums3[:, :, 67:68])
                Sg = sm.tile([P, S6], f32, tag="Sg")
                nc.scalar.copy(Sg[:, 0:1], smp[:, 0:1])
                nc.vector.tensor_tensor(
                    Sg[:, 1:], smp[:, 1:], smp[:, :S6 - 1], op=sub_op)
                # Z: per-segment sums of e, written j-innermost to align with S.
                Z = sm.tile([P, S6], f32, tag="Z")
                Z3 = Z[:].rearrange("p (r s) -> p r s", s=6)
                for j, (o, n) in enumerate(zip(OFFS, NVEC)):
                    nc.vector.tensor_reduce(
                        Z3[:, :, j:j + 1], e3[:, :, o:o + n], axis=AX, op=add_op)
                rz = sm.tile([P, S6], f32, tag="rz")
                nc.vector.reciprocal_approx_fast(rz[:], Z[:])
                # Fold inv_n into the reciprocal so one stt covers all 6 segs.
                rzi = sm.tile([P, S6], f32, tag="rzi")
                nc.vector.tensor_tensor(rzi[:], rz[:], invbb[:], op=mult_op)
                L = sm.tile([P, S6], f32, tag="L")
                for j in range(6):
                    nc.scalar.activation(
                        L[:, j * r:(j + 1) * r], Z3[:, :, j], Ln,
                        accum_out=accB[:, t * 6 + j: t * 6 + j + 1])
                to = sm.tile([P, S6], f32, tag="to")
                nc.vector.scalar_tensor_tensor(
                    out=to[:],
                    in0=Sg[:],
                    scalar=1.0,
                    in1=rzi[:],
                    op0=mult_op,
                    op1=mult_op,
                    accum_out=accA[:, t: t + 1])
            nc.sync.dma_start(acc_a[:], accA[:])
            nc.sync.dma_start(acc_b[:], accB[:])
    with _force_exp_ln_one_table_set():
        nc.compile()
    return nc, Tt


def _force_exp_ln_one_table_set():
    """Make the act-table pass map both Exp and Ln to
    natural_log_exp_and_others (otherwise it alternates exp_and_others /
    natural_log per tile: 14 ACT_TABLE_LOADs ~= 18us of ScalarE time)."""
    import contextlib
    import concourse.bacc as bacc_mod
    import concourse.mybir as mybir

    @contextlib.contextmanager
    def ctx():
        orig = bacc_mod.get_activation_tables

        def patched(arch):
            tables = {k: set(v) for k, v in orig(arch).items()}
            for name, funcs in tables.items():
                if name != "natural_log_exp_and_others":
                    funcs.discard(mybir.ActivationFunctionType.Exp)
                    funcs.discard(mybir.ActivationFunctionType.Ln)
            return tables

        bacc_mod.get_activation_tables = patched
        try:
            yield
        finally:
            bacc_mod.get_activation_tables = orig

    return ctx()


def _get_program():
    key = (W_CORE, R)
    if key not in _PROGRAM_CACHE:
        _PROGRAM_CACHE[key] = build_program(W_CORE, R)
    return _PROGRAM_CACHE[key]


def _host_x(current_action):
    """Segmented log_softmax of current_action in float64 on host."""
    ca = np.asarray(current_action, np.float64)
    x = np.empty(A, np.float64)
    for o, n in zip(OFFS, NVEC):
        seg = ca[o:o + n]
        m = seg.max()
        x[o:o + n] = seg - (m + np.log(np.exp(seg - m).sum()))
    return x


def combine_partials(results, w_full=W_FULL):
    """Combine per-core acc_a [P,T*4] (inv_n-weighted S/Z partials) and
    acc_b [P,T*6] (unweighted per-segment log-sums) into the scalar loss."""
    inv_n = 1.0 / np.asarray(NVEC, np.float64)
    total = 0.0
    for res in results:
        a = np.asarray(res["acc_a"], np.float64)
        b = np.asarray(res["acc_b"], np.float64)
        total += a.sum()  # inv_n already folded in on-device
        bt = b.reshape(P, -1, 6).sum(axis=(0, 1))  # [6] unweighted log-sums
        total -= (bt * inv_n).sum()
    return np.float32(total / w_full)


def _make_xbt(current_action):
    """Host-side xb payload: x broadcast [P, 68] ++ inv_n [P, 6]."""
    x = _host_x(current_action).astype(np.float32)
    row = np.concatenate([x, (1.0 / np.asarray(NVEC, np.float32))])
    return np.broadcast_to(row, (P, A + 6)).copy()


def kernel(current_action, previous_actions):
    from concourse import bass_utils

    nc, _ = _get_program()
    xbt = _make_xbt(current_action)
    pa = np.ascontiguousarray(np.asarray(previous_actions, np.float32))
    assert pa.shape == (W_FULL, A)
    in_maps = [
        {"pa": pa[c * W_CORE:(c + 1) * W_CORE], "xb": xbt}
        for c in range(N_CORES)
    ]
    res = bass_utils.run_bass_kernel_spmd(
        nc, in_maps, core_ids=list(range(N_CORES)))
    return combine_partials(res.results)


if __name__ == "__main__":
    np.random.seed(0)
    ca = np.random.randn(A).astype(np.float32)
    pa = np.random.randn(W_FULL, A).astype(np.float32)
    print(kernel(ca, pa))


# revision 15
# speedup vs baseline: 1.1709x; 1.1709x over previous
"""Trainium2 Bass kernel for nn_ActionSmoothingLoss.

Math (per row y of previous_actions, x = segmented log_softmax(current_action)):
    e = exp(y)                       (no max-subtraction: |y| <= ~5.5, safe in f32)
    Z_j = sum_{i in seg j} e_i
    S_j = sum_{i in seg j} e_i * (y_i - x_i)
    loss = (1/W) * sum_rows sum_j inv_n_j * [ S_j / Z_j - log Z_j ]

Sharding: data-parallel over W across 8 cores; x replicated; partial sums
(per-partition accumulators) gathered and combined on host.

Device pipeline per tile [128 partitions, r=64 rows x 68]:
    ScalarE: e = exp(y); per-segment Ln(Z) with accum into accB.
    DVE:     d = y - xbb (subtract)
             cums = MUL_CUMSUM(e, d)   <- custom DVE op: prefix-sum of the
                    product e*d in one pass (fuses multiply + S-reduction)
             smp  = cums sampled at the 6 segment-end columns of each row
             S    = adjacent-difference of smp  (exact per-(row,seg) sums:
                    the cumsum is continuous across rows, so diffs of
                    consecutive segment-end samples telescope correctly)
             Z    = 6 per-segment tensor_reduces over e
             rz   = reciprocal_approx_fast(Z)
             stt  = (S * inv_n) * rz with accum into accA (4 groups)
Final combine of accA/accB on host in float64.
"""

import sys

sys.path.insert(0, "/opt/trn_rl_repo")

import numpy as np

NVEC = (3, 3, 4, 25, 25, 8)
OFFS = (0, 3, 6, 10, 35, 60)
ENDS = (2, 5, 9, 34, 59, 67)  # inclusive end column of each segment
A = 68
P = 128
N_CORES = 8
W_FULL = 524288
W_CORE = W_FULL // N_CORES  # 65536
R = 64                      # rows per partition per tile
F = R * A                   # 4352 free elems per tile
T = W_CORE // (P * R)       # 8 tiles per core

_PROGRAM_CACHE = {}
_MUL_CUMSUM = None


def _register_mul_cumsum():
    """Register the MUL_CUMSUM_ANT custom DVE op (out = cumsum(in0*in1) along
    the free dim, fp32 state). Uses the documented extension point
    (dve_ops.OPS); the uop table ships inside the NEFF so no firmware change
    is involved. Idempotent."""
    global _MUL_CUMSUM
    if _MUL_CUMSUM is not None:
        return _MUL_CUMSUM
    import concourse.dve_ops as dve_ops_mod
    from concourse.dve_spec import Spec, Src0, Src1, AluOp, scan, lower
    from concourse.dve_uop import DveOpSpec

    NAME = "MUL_CUMSUM_ANT"
    for op in dve_ops_mod.OPS:
        if op.name == NAME:
            _MUL_CUMSUM = op
            return op

    def _ref(in0, in1, s0, s1, imm2):
        p = in0.shape[0]
        prod = (np.asarray(in0, np.float32).reshape(p, -1)
                * np.asarray(in1, np.float32).reshape(p, -1)).astype(np.float32)
        return np.cumsum(prod, axis=-1, dtype=np.float32)

    spec = Spec(body=scan(AluOp.ADD, Src0 * Src1), reference=_ref)
    row = dve_ops_mod._CUSTOM_DVE_ROW_BASE + len(dve_ops_mod.OPS)
    assert row < 0x20
    shas = {}
    for ver in ("v3",):
        s = DveOpSpec(name=NAME, opcode=row, uops=lower(spec, ver=ver), rd1_en=True)
        shas[ver] = s.sha(ver)
    op = dve_ops_mod.DveOp(NAME, spec, subdim=False, uops_sha=shas)
    dve_ops_mod.OPS.append(op)
    dve_ops_mod._SUB_OPCODE_FOR_NAME[NAME] = row
    dve_ops_mod.CUSTOM_DVE_SPECS[NAME] = spec
    _MUL_CUMSUM = op
    return op


def build_program(w_core=W_CORE, r=R):
    import concourse.bass as bass
    import concourse.bacc as bacc
    import concourse.mybir as mybir
    from concourse import tile

    mul_cumsum = _register_mul_cumsum()

    f32 = mybir.dt.float32
    bf16 = mybir.dt.bfloat16
    Ft = r * A
    S6 = 6 * r
    Tt = w_core // (P * r)
    assert Tt * P * r == w_core

    Exp = mybir.ActivationFunctionType.Exp
    Ln = mybir.ActivationFunctionType.Ln
    sub_op = mybir.AluOpType.subtract
    mult_op = mybir.AluOpType.mult
    add_op = mybir.AluOpType.add
    AX = mybir.AxisListType.X

    nc = bacc.Bacc(None, target_bir_lowering=False)
    pa = nc.dram_tensor("pa", [w_core, A], f32, kind="ExternalInput")
    # xb carries x broadcast (cols 0..67) plus the 6 inv_n values (68..73).
    xb = nc.dram_tensor("xb", [P, A + 6], f32, kind="ExternalInput")
    acc_a = nc.dram_tensor("acc_a", [P, Tt], f32, kind="ExternalOutput")
    acc_b = nc.dram_tensor("acc_b", [P, Tt * 4], f32, kind="ExternalOutput")

    pav = pa.rearrange("(t p r) a -> t p (r a)", t=Tt, p=P, r=r)

    with tile.TileContext(nc) as tc:
        with tc.tile_pool(name="io", bufs=3) as io, \
             tc.tile_pool(name="wk", bufs=2) as wk, \
             tc.tile_pool(name="sm", bufs=2) as sm, \
             tc.tile_pool(name="ps", bufs=1) as ps:
            xbt = ps.tile([P, A + 6], f32)
            nc.sync.dma_start(xbt[:], xb[:])
            accA = ps.tile([P, Tt], f32)
            accB = ps.tile([P, Tt * 4], f32)
            # x broadcast to [P, r*A] once (sub's in1 per half).
            xbb = ps.tile([P, Ft], f32)
            nc.vector.tensor_copy(
                xbb[:].rearrange("p (r a) -> p r a", r=r),
                xbt[:, :A].unsqueeze(1).broadcast_to((P, r, A)))
            # inv_n broadcast to the [P, r, 6] j-innermost layout once.
            invbb = ps.tile([P, 6 * r], f32)
            nc.vector.tensor_copy(
                invbb[:].rearrange("p (r s) -> p r s", s=6),
                xbt[:, A:A + 6].unsqueeze(1).broadcast_to((P, r, 6)))
            H = Ft // 2
            for t in range(Tt):
                y = io.tile([P, Ft], f32, tag="y")
                nc.sync.dma_start(y[:, :H], pav[t][:, :H])
                nc.sync.dma_start(y[:, H:], pav[t][:, H:])
                # e/d in bf16: halves SBUF streaming (cross-engine bank
                # contention measurably inflates DVE op durations). xbb stays
                # fp32 — its quantization error is common-mode across rows and
                # would bias the loss (~1.4e-3); e/d rounding averages out.
                e = wk.tile([P, Ft], bf16, tag="e")
                d = wk.tile([P, Ft], bf16, tag="d")
                cums = wk.tile([P, Ft], f32, tag="cums")
                for h in (slice(0, H), slice(H, Ft)):
                    nc.scalar.activation(e[:, h], y[:, h], Exp)
                    nc.vector.tensor_tensor(d[:, h], y[:, h], xbb[:, h], op=sub_op)
                e3 = e[:].rearrange("p (r a) -> p r a", r=r)
                d3 = d[:].rearrange("p (r a) -> p r a", r=r)
                # cums = running sum of e*d over the flat [r*A] stream.
                nc.vector._custom_dve(
                    mul_cumsum, out=cums[:], in0=e3, in1=d3)
                cums3 = cums[:].rearrange("p (r a) -> p r a", r=r)
                # Sample the cumsum at each segment-end column; j-innermost
                # layout so one adjacent-difference yields every segment sum.
                # End cols {2,5}, {9,34,59}, {67} have affine strides, so three
                # strided copies cover all six.
                smp = sm.tile([P, S6], f32, tag="smp")
                smp3 = smp[:].rearrange("p (r s) -> p r s", s=6)
                nc.vector.tensor_copy(smp3[:, :, 0:2], cums3[:, :, 2:6:3])
                nc.vector.tensor_copy(smp3[:, :, 2:5], cums3[:, :, 9:60:25])
                nc.vector.tensor_copy(smp3[:, :, 5:6], cums3[:, :, 67:68])
                Sg = sm.tile([P, S6], f32, tag="Sg")
                nc.vector.tensor_copy(Sg[:, 0:1], smp[:, 0:1])
                nc.vector.tensor_tensor(
                    Sg[:, 1:], smp[:, 1:], smp[:, :S6 - 1], op=sub_op)
                # Z: per-segment sums of e, written j-innermost to align with S.
                Z = sm.tile([P, S6], f32, tag="Z")
                Z3 = Z[:].rearrange("p (r s) -> p r s", s=6)
                for j, (o, n) in enumerate(zip(OFFS, NVEC)):
                    nc.vector.tensor_reduce(
                        Z3[:, :, j:j + 1], e3[:, :, o:o + n], axis=AX, op=add_op)
                rz = sm.tile([P, S6], f32, tag="rz")
                nc.vector.reciprocal_approx_fast(rz[:], Z[:])
                # Fold inv_n into the reciprocal so one stt covers all 6 segs.
                rzi = sm.tile([P, S6], f32, tag="rzi")
                nc.vector.tensor_tensor(rzi[:], rz[:], invbb[:], op=mult_op)
                # Ln accumulation grouped by equal inv_n (segments {0,1}, {2},
                # {3,4}, {5}) — 4 ScalarE instrs instead of 6.
                L = sm.tile([P, S6], f32, tag="L")
                for g, (j0, k) in enumerate(((0, 2), (2, 1), (3, 2), (5, 1))):
                    nc.scalar.activation(
                        L[:, j0 * r:(j0 + k) * r].rearrange(
                            "p (r s) -> p r s", s=k),
                        Z3[:, :, j0:j0 + k], Ln,
                        accum_out=accB[:, t * 4 + g: t * 4 + g + 1])
                to = sm.tile([P, S6], f32, tag="to")
                nc.vector.scalar_tensor_tensor(
                    out=to[:],
                    in0=Sg[:],
                    scalar=1.0,
                    in1=rzi[:],
                    op0=mult_op,
                    op1=mult_op,
                    accum_out=accA[:, t: t + 1])
            nc.sync.dma_start(acc_a[:], accA[:])
            nc.sync.dma_start(acc_b[:], accB[:])
    with _force_exp_ln_one_table_set():
        nc.compile()
    return nc, Tt


def _force_exp_ln_one_table_set():
    """Make the act-table pass map both Exp and Ln to
    natural_log_exp_and_others (otherwise it alternates exp_and_others /
    natural_log per tile: 14 ACT_TABLE_LOADs ~= 18us of ScalarE time)."""
    import contextlib
    import concourse.bacc as bacc_mod
    import concourse.mybir as mybir

    @contextlib.contextmanager
    def ctx():
        orig = bacc_mod.get_activation_tables

        def patched(arch):
            tables = {k: set(v) for k, v in orig(arch).items()}
            for name, funcs in tables.items():
                if name != "natural_log_exp_and_others":
                    funcs.discard(mybir.ActivationFunctionType.Exp)
                    funcs.discard(mybir.ActivationFunctionType.Ln)
            return tables

        bacc_mod.get_activation_tables = patched
        try:
            yield
        finally:
            bacc_mod.get_activation_tables = orig

    return ctx()


def _get_program():
    key = (W_CORE, R)
    if key not in _PROGRAM_CACHE:
        _PROGRAM_CACHE[key] = build_program(W_CORE, R)
    return _PROGRAM_CACHE[key]


def _host_x(current_action):
    """Segmented log_softmax of current_action in float64 on host."""
    ca = np.asarray(current_action, np.float64)
    x = np.empty(A, np.float64)
    for o, n in zip(OFFS, NVEC):
        seg = ca[o:o + n]
        m = seg.max()
        x[o:o + n] = seg - (m + np.log(np.exp(seg - m).sum()))
    return x


def combine_partials(results, w_full=W_FULL):
    """Combine per-core acc_a [P,T] (inv_n-weighted S/Z partials) and
    acc_b [P,T*4] (per-inv_n-group log-sums) into the scalar loss."""
    inv_g = np.asarray([1.0 / 3, 1.0 / 4, 1.0 / 25, 1.0 / 8], np.float64)
    total = 0.0
    for res in results:
        a = np.asarray(res["acc_a"], np.float64)
        b = np.asarray(res["acc_b"], np.float64)
        total += a.sum()  # inv_n already folded in on-device
        bt = b.reshape(P, -1, 4).sum(axis=(0, 1))  # [4] group log-sums
        total -= (bt * inv_g).sum()
    return np.float32(total / w_full)


def _make_xbt(current_action):
    """Host-side xb payload: x broadcast [P, 68] ++ inv_n [P, 6]."""
    x = _host_x(current_action).astype(np.float32)
    row = np.concatenate([x, (1.0 / np.asarray(NVEC, np.float32))])
    return np.broadcast_to(row, (P, A + 6)).copy()


def kernel(current_action, previous_actions):
    from concourse import bass_utils

    nc, _ = _get_program()
    xbt = _make_xbt(current_action)
    pa = np.ascontiguousarray(np.asarray(previous_actions, np.float32))
    assert pa.shape == (W_FULL, A)
    in_maps = [
        {"pa": pa[c * W_CORE:(c + 1) * W_CORE], "xb": xbt}
        for c in range(N_CORES)
    ]
    res = bass_utils.run_bass_kernel_spmd(
        nc, in_maps, core_ids=list(range(N_CORES)))
    return combine_partials(res.results)


if __name__ == "__main__":
    np.random.seed(0)
    ca = np.random.randn(A).astype(np.float32)
    pa = np.random.randn(W_FULL, A).astype(np.float32)
    print(kernel(ca, pa))
